# revision 13
# baseline (speedup 1.0000x reference)
"""MetaSR Trainium2 kernel (8 NeuronCores, SPMD).

Sharding: core = (batch b, query-half). Each core computes the 3x3 conv
encoder for its batch (redundant x2), then processes 8192 queries:
nearest-index math -> dma_gather of 3x3x64 neighborhoods -> reordered
big matmul v = q_feat @ U (f32r on PE) -> DVE products with hdn ->
PE ones-reduce to pred.

Math reordering: pred[q,c] = sum_h hdn[q,h] * v[q,h,c] + bias_c(q)
where v[q,:,:] = q_feat[q,:] @ U, U[k,(c,h)] = fc2_w[h, kref(k)*3+c].
This keeps the big (Q x 576 x 771) contraction weight-stationary on PE.

Within-core query order: q = 64*p + s (p=partition, s=0..63); all
on-chip column streams use j = 128*s + p; host unshards at the end.
"""

import numpy as np
from contextlib import ExitStack

import concourse.bass as bass
import concourse.mybir as mybir
import concourse.tile as tile
from concourse import bacc
from concourse.bass_utils import run_bass_kernel_spmd
from concourse.masks import make_identity

F32 = mybir.dt.float32
F32R = mybir.dt.float32r
I16 = mybir.dt.int16

B, C_IN, H, W = 4, 3, 128, 128
C = 64
Q = 16384
HID = 256
NCORES = 8
NQ = Q * B // NCORES          # 8192 queries per core
PW = H + 2                    # 130 padded width
NPOS = PW * PW                # 16900
KTOT = C * 9                  # 576
MTOT = 3 * HID + 3            # 771 output cols of U
NQT = 512                     # compute tile (queries)
NTILES = NQ // NQT            # 16
QG = 1024                     # gather slab (queries)
NSLABS = NQ // QG             # 8
SPT = NQT // 128              # 4 s-values per tile
MAGIC = 12582912.0            # 1.5 * 2**23, fp32 round-to-nearest-even

# k-chunking of the 576 gather rows, per-ky aligned so every PE transpose
# writes PSUM partition 0 (walrus requires transpose outputs at partition 0):
# chunk 2*ky = ky[0:128], chunk 2*ky+1 = ky[128:192]
KCH = [128, 64, 128, 64, 128, 64]
KSTART = [0, 128, 192, 320, 384, 512]
NKC = len(KCH)
# transposes per query-block: (ky, lo, hi, kc)
TR_PLAN = [
    (0, 0, 128, 0),
    (0, 128, 192, 1),
    (1, 0, 128, 2),
    (1, 128, 192, 3),
    (2, 0, 128, 4),
    (2, 128, 192, 5),
]


def _build():
    nc = bacc.Bacc("TRN2", target_bir_lowering=False, debug=False,
                   num_devices=NCORES)

    inp_d = nc.dram_tensor("inp", [C_IN, H * W], F32, kind="ExternalInput").ap()
    coord_d = nc.dram_tensor("coord", [NQ, 2], F32, kind="ExternalInput").ap()
    cell_d = nc.dram_tensor("cell", [NQ, 2], F32, kind="ExternalInput").ap()
    convw_d = nc.dram_tensor("convw", [27, C], F32, kind="ExternalInput").ap()
    convb_d = nc.dram_tensor("convb", [128, C], F32, kind="ExternalInput").ap()
    fc1t_d = nc.dram_tensor("fc1t", [4, HID], F32, kind="ExternalInput").ap()
    u_d = nc.dram_tensor("u", [KTOT, MTOT], F32, kind="ExternalInput").ap()
    sel_d = nc.dram_tensor("sel", [128, 9], F32, kind="ExternalInput").ap()
    out_d = nc.dram_tensor("out", [3, NQ], F32, kind="ExternalOutput").ap()

    # internal DRAM scratch
    pad_t = nc.dram_tensor("pad_img", [C_IN * NPOS + 116], F32, kind="Internal")
    tab_t = nc.dram_tensor("feat_tab", [NPOS * C + 128], F32, kind="Internal")
    pad_d = pad_t.ap()
    tab_d = tab_t.ap()

    with tile.TileContext(nc) as tc, ExitStack() as ctx:
        cpool = ctx.enter_context(tc.tile_pool(name="consts", bufs=1))

        # ---- constants in SBUF ----
        ident = cpool.tile([128, 128], F32, tag="ident")
        make_identity(nc, ident[:])

        u_sb = cpool.tile([128, NKC, MTOT], F32R, tag="u")
        fc1t_sb = cpool.tile([4, HID], F32R, tag="fc1t")
        sel_sb = cpool.tile([128, 9], F32R, tag="sel")
        with tc.tile_pool(name="cstage", bufs=1) as spool:
            u_f32 = spool.tile([128, NKC, MTOT], F32, tag="uf")
            for kc in range(NKC):
                st, sz = KSTART[kc], KCH[kc]
                nc.sync.dma_start(u_f32[0:sz, kc, :], u_d[st:st + sz, :])
                nc.vector.tensor_copy(u_sb[0:sz, kc, :], u_f32[0:sz, kc, :])

            fc1t_f = spool.tile([4, HID], F32, tag="fc1tf")
            nc.sync.dma_start(fc1t_f[:], fc1t_d[:])
            nc.vector.tensor_copy(fc1t_sb[:], fc1t_f[:])

            sel_f = spool.tile([128, 9], F32, tag="self")
            nc.sync.dma_start(sel_f[:], sel_d[:])
            nc.vector.tensor_copy(sel_sb[:], sel_f[:])

        convw_sb = cpool.tile([27, C], F32, tag="convw")
        nc.sync.dma_start(convw_sb[:], convw_d[:])
        convb_sb = cpool.tile([128, C], F32, tag="convb")
        nc.sync.dma_start(convb_sb[:], convb_d[:])

        zeros_sb = cpool.tile([128, 512], F32, tag="zeros")
        nc.vector.memset(zeros_sb[:], 0.0)

        # =========== STAGE 1: conv encoder -> DRAM feat table ===========
        with tc.tile_pool(name="convp", bufs=1) as convpool, \
             tc.tile_pool(name="convps", bufs=2, space="PSUM") as convps:
            # zero-fill padded image (50816 = 128*397)
            nc.sync.dma_start(
                pad_d[0:50816].rearrange("(p f) -> p f", p=128),
                zeros_sb[:, 0:397])
            # copy inp into interior of padded image
            nc.sync.dma_start(
                bass.AP(pad_t, PW + 1, [[NPOS, C_IN], [PW, H], [1, W]]),
                bass.AP(inp_d.tensor, 0, [[H * W, C_IN], [W, H], [1, W]]))
            # feat table borders + tail: zero
            nc.sync.dma_start(
                tab_d[0:PW * C].rearrange("(p f) -> p f", p=128),
                zeros_sb[:, 0:65])
            nc.sync.dma_start(
                tab_d[(NPOS - PW) * C:(NPOS - PW) * C + 128 * 66]
                .rearrange("(p f) -> p f", p=128),
                zeros_sb[:, 0:66])
            nc.sync.dma_start(
                bass.AP(tab_t, PW * C, [[PW * C, 128], [1, C]]),
                zeros_sb[:, 0:C])
            nc.sync.dma_start(
                bass.AP(tab_t, PW * C + 129 * C, [[PW * C, 128], [1, C]]),
                zeros_sb[:, 0:C])

            # im2col [27, 16384]
            im2col = convpool.tile([27, H * W], F32, tag="im2col")
            for c in range(C_IN):
                for ky in range(3):
                    for kx in range(3):
                        r = c * 9 + ky * 3 + kx
                        nc.sync.dma_start(
                            im2col[r:r + 1, :],
                            bass.AP(pad_t, c * NPOS + ky * PW + kx,
                                    [[PW, H], [1, W]]))

            # conv matmuls: px-chunk = one y-row (128 px), out [128 x, 64 ch]
            feat_sb = convpool.tile([128, H, C], F32, tag="feat")
            for y in range(H):
                ps = convps.tile([128, C], F32, tag="cps")
                nc.tensor.matmul(ps[:], im2col[:, y * 128:(y + 1) * 128],
                                 convw_sb[:], start=True, stop=True)
                nc.vector.tensor_tensor(feat_sb[:, y, :], ps[:], convb_sb[:],
                                        mybir.AluOpType.add)
            # write interior of feat table: row (y+1)*130 + (x+1)
            nc.sync.dma_start(
                bass.AP(tab_t, (PW + 1) * C, [[C, 128], [PW * C, H], [1, C]]),
                feat_sb[:])

        # =========== STAGE 2: coordinate math + gather index tables =====
        mpool = ctx.enter_context(tc.tile_pool(name="meta", bufs=1))
        crd = mpool.tile([128, 64, 2], F32, tag="crd")
        cel = mpool.tile([128, 64, 2], F32, tag="cel")
        nc.sync.dma_start(crd[:], coord_d.rearrange("(p s) two -> p s two", p=128))
        nc.sync.dma_start(cel[:], cell_d.rearrange("(p s) two -> p s two", p=128))

        mlp = mpool.tile([128, 64, 4], F32, tag="mlp")
        lin16 = mpool.tile([128, 3, 64], I16, tag="lin16")
        t1 = mpool.tile([128, 64], F32, tag="t1")
        t2 = mpool.tile([128, 64], F32, tag="t2")
        cq = mpool.tile([128, 64], F32, tag="cq")
        ii = {}
        for ax, slot in ((0, 0), (1, 1)):  # ax 0 = y, 1 = x
            co = crd[:, :, ax]
            ce = cel[:, :, ax]
            # coord_ = coord - cell/2
            nc.vector.tensor_scalar_mul(t1[:], ce, 0.5)
            nc.vector.tensor_tensor(t2[:], co, t1[:], mybir.AluOpType.subtract)
            # cq = clip(coord_ + eps, -1+eps, 1-eps)
            nc.vector.tensor_scalar_add(cq[:], t2[:], 1e-6)
            nc.vector.tensor_scalar(cq[:], cq[:], 1.0 - 1e-6, -1.0 + 1e-6,
                                    mybir.AluOpType.min, mybir.AluOpType.max)
            # v = ((cq + 1)*128 - 1)/2 ; iy = clip(rne(v), 0, 127)
            nc.vector.tensor_scalar_add(cq[:], cq[:], 1.0)
            nc.vector.tensor_scalar_mul(cq[:], cq[:], float(H))
            nc.vector.tensor_scalar_add(cq[:], cq[:], -1.0)
            nc.vector.tensor_scalar_mul(cq[:], cq[:], 0.5)
            nc.vector.tensor_scalar_add(cq[:], cq[:], MAGIC)
            nc.vector.tensor_scalar_add(cq[:], cq[:], -MAGIC)
            nc.vector.tensor_scalar(cq[:], cq[:], 127.0, 0.0,
                                    mybir.AluOpType.min, mybir.AluOpType.max)
            idx = mpool.tile([128, 64], F32, tag=f"idx{ax}")
            nc.vector.tensor_copy(idx[:], cq[:])
            ii[ax] = idx
            # qc = idx/64 - 1 ; rel = (coord_ - qc) * 64
            nc.vector.tensor_scalar(t1[:], idx[:], 1.0 / 64.0, -1.0,
                                    mybir.AluOpType.mult, mybir.AluOpType.add)
            nc.vector.tensor_tensor(t1[:], t2[:], t1[:], mybir.AluOpType.subtract)
            nc.vector.tensor_scalar_mul(mlp[:, :, slot], t1[:], 64.0)
        # r_rev = cell_y * 64 ; ones
        nc.vector.tensor_scalar_mul(mlp[:, :, 2], cel[:, :, 0], 64.0)
        nc.vector.memset(mlp[:, :, 3], 1.0)

        # lin = (iy + ky)*130 + ix  -> int16
        for ky in range(3):
            nc.vector.tensor_scalar(t1[:], ii[0][:], float(ky), float(PW),
                                    mybir.AluOpType.add, mybir.AluOpType.mult)
            nc.vector.tensor_tensor(t1[:], t1[:], ii[1][:], mybir.AluOpType.add)
            nc.vector.tensor_copy(lin16[:, ky, :], t1[:])

        # fold [128,3,64] -> lin2 [16, 3, 8, 64] -> idx_tab [128, 3, 512]
        lin2 = mpool.tile([16, 3, 8, 64], I16, tag="lin2")
        idx_tab = mpool.tile([128, 3, 512], I16, tag="idxtab")
        nc.vector.memset(idx_tab[:], 0)
        for ph in range(8):
            nc.sync.dma_start(lin2[0:16, :, ph, :],
                              lin16[ph * 16:(ph + 1) * 16, :, :])
        for ky in range(3):
            nc.vector.tensor_copy(
                idx_tab[0:16, ky, :].rearrange("p (s ph) -> p ph s", ph=8),
                lin2[0:16, ky, :, :])
        # each GPSIMD Q7 core reads its own 16-partition replica of the idxs
        for r in range(1, 8):
            nc.sync.dma_start(idx_tab[16 * r:16 * (r + 1), :, :],
                              idx_tab[0:16, :, :])

        # mlp_in transposes: [128,4] per s -> mlpT [4, 8192] (f32r)
        mlpT = mpool.tile([4, NQ], F32R, tag="mlpT")
        with tc.tile_pool(name="mtps", bufs=2, space="PSUM") as mtps:
            for s in range(64):
                ps = mtps.tile([4, 128], F32, tag="mt")
                nc.tensor.transpose(ps[:], mlp[:, s, :], ident[:])
                nc.vector.tensor_copy(mlpT[:, s * 128:(s + 1) * 128], ps[:])

        # =========== STAGE 3: per-tile pipeline ===========
        # gather source AP: overlapping 192-elem windows, stride 64
        gsrc = bass.AP(tab_t, 0, [[C, NPOS - 2], [1, 192]])

        gpool = ctx.enter_context(tc.tile_pool(name="gat", bufs=2))
        qpool = ctx.enter_context(tc.tile_pool(name="qf", bufs=2))
        hpool = ctx.enter_context(tc.tile_pool(name="hdn", bufs=2))
        ppool = ctx.enter_context(tc.tile_pool(name="prod", bufs=3))
        opool = ctx.enter_context(tc.tile_pool(name="outp", bufs=2))
        ps_v = ctx.enter_context(tc.tile_pool(name="psv", bufs=3, space="PSUM"))
        ps_sm = ctx.enter_context(tc.tile_pool(name="pssm", bufs=2, space="PSUM"))
        ps_pr = ctx.enter_context(tc.tile_pool(name="pspr", bufs=1, space="PSUM"))
        ps_tr = ctx.enter_context(tc.tile_pool(name="pstr", bufs=2, space="PSUM"))

        gats = {}
        for t in range(NTILES):
            g = t // 2
            if t % 2 == 0:
                # issue gathers for slab g
                gat = gpool.tile([128, 3, QG // 128, 192], F32, tag="gat")
                for ky in range(3):
                    nc.gpsimd.dma_gather(
                        gat[:, ky, :, :], gsrc,
                        idx_tab[:, ky, g * (QG // 16):(g + 1) * (QG // 16)],
                        num_idxs=QG, num_idxs_reg=QG,
                        elem_size=192, elem_step=C)
                gats[g] = gat
            gat = gats[g]

            # fc1: hdnT [128, 2, 512]
            hdnT = hpool.tile([128, 2, NQT], F32, tag="hdnT")
            for hc in range(2):
                ps_h = ps_sm.tile([128, NQT], F32, tag="sm")
                nc.tensor.matmul(ps_h[:], fc1t_sb[:, hc * 128:(hc + 1) * 128],
                                 mlpT[:, t * NQT:(t + 1) * NQT],
                                 start=True, stop=True)
                nc.scalar.activation(hdnT[:, hc, :], ps_h[:],
                                     mybir.ActivationFunctionType.Relu)

            # q_feat^T tiles [128, 5, 512] f32r via PE transposes
            qfT = qpool.tile([128, NKC, NQT], F32R, tag="qfT")
            for qb in range(SPT):
                sl = (t % 2) * SPT + qb  # s_local within slab
                for (ky, lo, hi, kc) in TR_PLAN:
                    n = hi - lo
                    trp = ps_tr.tile([128, 128], F32, tag="tr")
                    nc.tensor.transpose(
                        trp[0:n, :], gat[:, ky, sl, lo:hi], ident[:])
                    nc.vector.tensor_copy(
                        qfT[0:n, kc, qb * 128:(qb + 1) * 128], trp[0:n, :])

            # big matmul: v chunks + products + ones-reduce
            ps_pred = ps_pr.tile([4, NQT], F32, tag="pred")
            for mc in range(6):
                c, hh = mc // 2, mc % 2
                ps_vt = ps_v.tile([128, NQT], F32, tag="v")
                for kc in range(NKC):
                    kk = KCH[kc]
                    nc.tensor.matmul(
                        ps_vt[:], u_sb[0:kk, kc, mc * 128:(mc + 1) * 128],
                        qfT[0:kk, kc, :], start=(kc == 0), stop=(kc == NKC - 1))
                prod = ppool.tile([128, NQT], F32R, tag="P")
                nc.vector.tensor_tensor(prod[:], hdnT[:, hh, :], ps_vt[:],
                                        mybir.AluOpType.mult)
                nc.tensor.matmul(ps_pred[0:3, :], sel_sb[:, c * 3:(c + 1) * 3],
                                 prod[:], start=(mc == 0), stop=False)
            # bias chunk (U cols 768:771) accumulates into the same psum
            for kc in range(NKC):
                kk = KCH[kc]
                nc.tensor.matmul(ps_pred[0:3, :], u_sb[0:kk, kc, 768:771],
                                 qfT[0:kk, kc, :], start=False,
                                 stop=(kc == NKC - 1))
            pred_sb = opool.tile([3, NQT], F32, tag="pred")
            nc.vector.tensor_copy(pred_sb[:], ps_pred[0:3, :])
            nc.sync.dma_start(out_d[:, t * NQT:(t + 1) * NQT], pred_sb[:])

    nc.compile()
    return nc


_NC = None


def _get_nc():
    global _NC
    if _NC is None:
        _NC = _build()
    return _NC


def _prep_in_maps(inp, coord, cell, conv_w, conv_b, fc1_w, fc1_b, fc2_w, fc2_b):
    inp = np.asarray(inp, np.float32)
    coord = np.asarray(coord, np.float32)
    cell = np.asarray(cell, np.float32)
    conv_w = np.asarray(conv_w, np.float32)
    conv_b = np.asarray(conv_b, np.float32)
    fc1_w = np.asarray(fc1_w, np.float32)
    fc1_b = np.asarray(fc1_b, np.float32)
    fc2_w = np.asarray(fc2_w, np.float32)
    fc2_b = np.asarray(fc2_b, np.float32)

    convw = np.ascontiguousarray(
        conv_w.transpose(1, 2, 3, 0).reshape(27, C))          # [27, 64]
    convb = np.ascontiguousarray(np.tile(conv_b[None, :], (128, 1)))
    fc1t = np.ascontiguousarray(
        np.concatenate([fc1_w, fc1_b[None, :]], 0))           # [4, 256]

    w2 = fc2_w.reshape(HID, C * 9, 3)
    b2 = fc2_b.reshape(C * 9, 3)
    ky, kx, ch = np.meshgrid(np.arange(3), np.arange(3), np.arange(C),
                             indexing="ij")
    perm = (ch * 9 + ky * 3 + kx).reshape(-1)                 # [576]
    U = np.zeros((KTOT, MTOT), np.float32)
    U[:, :768] = w2[:, perm, :].transpose(1, 2, 0).reshape(KTOT, 768)
    U[:, 768:771] = b2[perm, :]
    U = np.ascontiguousarray(U)

    sel = np.zeros((128, 9), np.float32)
    for c in range(3):
        sel[:, c * 3 + c] = 1.0

    in_maps = []
    for core in range(NCORES):
        b, half = core // 2, core % 2
        in_maps.append({
            "inp": np.ascontiguousarray(inp[b].reshape(C_IN, H * W)),
            "coord": np.ascontiguousarray(coord[b, half * NQ:(half + 1) * NQ]),
            "cell": np.ascontiguousarray(cell[b, half * NQ:(half + 1) * NQ]),
            "convw": convw, "convb": convb, "fc1t": fc1t, "u": U, "sel": sel,
        })
    return in_maps


def _unshard(results):
    out = np.zeros((B, Q, 3), np.float32)
    for core in range(NCORES):
        b, half = core // 2, core % 2
        pt = results[core]["out"]                             # [3, 8192]
        blk = pt.reshape(3, 64, 128).transpose(2, 1, 0).reshape(NQ, 3)
        out[b, half * NQ:(half + 1) * NQ] = blk
    return out


def run_spmd(in_maps, **kwargs):
    nc = _get_nc()
    return run_bass_kernel_spmd(nc, in_maps, core_ids=list(range(NCORES)),
                                **kwargs)


def kernel(**inputs) -> np.ndarray:
    in_maps = _prep_in_maps(**inputs)
    res = run_spmd(in_maps)
    return _unshard(res.results)
